# revision 90
# baseline (speedup 1.0000x reference)
"""Bass/Tile MHA kernel for TRN2 — per-core program (v3).

Sharding (8 cores): core c handles batch b=c//2, head-group g=c%2
(8 of 16 heads).  Host pre-transposes x per batch to xT [1024, 2048]
and provides it (and the x32-scaled W_q/k/v head-group slices) as
fp8e4m3 hi/lo pairs for DoubleRow matmuls; folds bv into
bo' = bo + bv @ Wo (softmax weights sum to 1); drops bk entirely
(a key-side bias adds a per-query constant to every logit, which
softmax cancels).  The x32 weight scale keeps the fp8 lo-residual out
of the e4m3 subnormal floor; 1/1024 folds into the exp scale and 1/32
into wo.

Per core inputs:
  xh/xl [1024,2048] fp8, w{q,k,v}{h,l} [1024,512] fp8 (x32, col
  slice), wo [512,1024] bf16 (row slice, /32), bqt [128,4] f32 (x32),
  masks [128,512] bf16 (diag dd=0 | dd=128 for 256-wide q chunks),
  ident [128,128] bf16.  out [2048,1024] f32 partial (host sums TP
  pairs + bo').

Per-core schedule:
  B(sc): DMA x s-chunk -> qT[j,s] (+bq), kT[j,s], v pair blocks via
    fp8 DoubleRow (3 hi/lo products per dc-pair, 0.75 cyc/row/dc).
  C(hp,qc): per head pair / 256-q chunk / head: scores K^T.Q per
    128-k causal tile (bf16, masked diagonal tiles first, top tile's
    dead 128-q half skipped), exp (ACT -> bf16), attnV FLIPPED:
    po[q, V|1] with q on all 128 partitions so the softmax denominator
    lands on the numerators' partition -> per-partition recip+scale
    (no PE broadcast).
  D(st): PE-transpose o[q,dv] -> oT (bf16 identity), out = oT.wo.
  The loop interleaves B/D pieces into C's ACT-bound stretches via a
  credit-paced filler queue with data-dependency require() guards.
"""

from contextlib import ExitStack

import numpy as np

import concourse.bass as bass
import concourse.bacc as bacc
import concourse.mybir as mybir
import concourse.tile as tile

F32 = mybir.dt.float32
BF16 = mybir.dt.bfloat16
FP8 = mybir.dt.float8e4
DR = mybir.MatmulPerfMode.DoubleRow
ADD = mybir.AluOpType.add
MULT = mybir.AluOpType.mult
EXP = mybir.ActivationFunctionType.Exp

S = 2048          # sequence length
D = 1024          # model dim
JC = 512          # per-core projection width (8 heads * 64)
DK = 64           # head dim
NSC = 4           # s-chunks of 512
NST = 16          # s-tiles of 128
ND = 8            # d-chunks of 128
NJ = 4            # j-chunks of 128 (head pairs)
NQC = 8           # q-chunks of 256
QW = 256          # q-chunk width in phase C
VW = DK + 1       # 65: even-head attnV window (V + ones col)
PW = 160          # v pair block: [V_even(64)|ones(1)|gap(31)|V_odd(64)]


def build_mha():
    nc = bacc.Bacc("TRN2", target_bir_lowering=False, debug=False)

    xh = nc.dram_tensor("xh", [D, S], FP8, kind="ExternalInput").ap()
    xl = nc.dram_tensor("xl", [D, S], FP8, kind="ExternalInput").ap()
    wqh = nc.dram_tensor("wqh", [D, JC], FP8, kind="ExternalInput").ap()
    wql = nc.dram_tensor("wql", [D, JC], FP8, kind="ExternalInput").ap()
    wkh = nc.dram_tensor("wkh", [D, JC], FP8, kind="ExternalInput").ap()
    wkl = nc.dram_tensor("wkl", [D, JC], FP8, kind="ExternalInput").ap()
    wvh = nc.dram_tensor("wvh", [D, JC], FP8, kind="ExternalInput").ap()
    wvl = nc.dram_tensor("wvl", [D, JC], FP8, kind="ExternalInput").ap()
    wo = nc.dram_tensor("wo", [JC, D], BF16, kind="ExternalInput").ap()
    bqt = nc.dram_tensor("bqt", [128, NJ], F32, kind="ExternalInput").ap()
    masks = nc.dram_tensor("masks", [128, 2 * QW], BF16,
                           kind="ExternalInput").ap()
    ident = nc.dram_tensor("ident", [128, 128], BF16,
                           kind="ExternalInput").ap()
    out = nc.dram_tensor("out", [S, D], BF16,
                         kind="ExternalOutput").ap()

    xhd3 = xh.rearrange("(c p) s -> p c s", p=128)
    xld3 = xl.rearrange("(c p) s -> p c s", p=128)

    with tile.TileContext(nc) as tc, ExitStack() as ctx:
        const = ctx.enter_context(tc.tile_pool(name="const", bufs=1))
        bq_t = const.tile([128, NJ], F32)
        ident_t = const.tile([128, 128], BF16)
        mask_t = const.tile([128, 2 * QW], BF16)

        # persistent activations
        big = ctx.enter_context(tc.tile_pool(name="big", bufs=1))
        qT_t = big.tile([128, NJ * S], BF16, tag="qT")    # [j, s] 16KB/part
        kT_t = big.tile([128, NJ * S], BF16, tag="kT")    # [j, s] 16KB/part
        v_t = big.tile([128, NST * 4 * PW], BF16, tag="v")  # [s, pair*160]
        o_sb = big.tile([128, NST * NJ * 128], BF16, tag="o")  # [q,(qt hp d)]

        # psum pools (8 banks: pps 2*1 + psc 2*2 + pat 2*1)
        pps = ctx.enter_context(tc.tile_pool(name="pps", bufs=2, space="PSUM"))
        psc = ctx.enter_context(tc.tile_pool(name="psc", bufs=2, space="PSUM"))
        pat = ctx.enter_context(tc.tile_pool(name="pat", bufs=2, space="PSUM"))

        ep = ctx.enter_context(tc.tile_pool(name="exp", bufs=6))
        rp = ctx.enter_context(tc.tile_pool(name="recip", bufs=4))
        otp = ctx.enter_context(tc.tile_pool(name="otp", bufs=2))

        wp = ctx.enter_context(tc.tile_pool(name="wts", bufs=1))
        xin = ctx.enter_context(tc.tile_pool(name="xin", bufs=2))
        w_ts = {}
        for wname in ("wqh", "wql", "wkh", "wkl", "wvh", "wvl"):
            w_ts[wname] = wp.tile([128, ND * JC], FP8, tag=wname,
                                  name=wname)
        wo_t = wp.tile([128, NJ * D], BF16, tag="wo")

        ostp = ctx.enter_context(tc.tile_pool(name="ost", bufs=4))

        xc_tiles = {}

        def dma_xc(sc, half=None):
            if half is None or half == 0:
                xch = xin.tile([128, ND * 512], FP8, tag="xch", name="xch")
                xcl = xin.tile([128, ND * 512], FP8, tag="xcl", name="xcl")
                xc_tiles[sc] = (
                    xch[:].rearrange("p (c s) -> p c s", c=ND),
                    xcl[:].rearrange("p (c s) -> p c s", c=ND))
            hd = ND // 2
            halves = range(2) if half is None else (half,)
            for h in halves:
                for xcr, xd3 in zip(xc_tiles[sc], (xhd3, xld3)):
                    nc.sync.dma_start(
                        xcr[:, h * hd:(h + 1) * hd, :],
                        xd3[:, h * hd:(h + 1) * hd,
                            sc * 512:(sc + 1) * 512])

        def dma_w(wname, w_dram, half):
            hd = ND // 2
            nc.sync.dma_start(
                w_ts[wname][:].rearrange("p (c j) -> p c j", c=ND)[
                    :, half * hd:(half + 1) * hd, :],
                w_dram.rearrange("(c p) j -> p c j", p=128)[
                    :, half * hd:(half + 1) * hd, :])

        # ---- startup DMAs (emission order = fetch priority) ----
        dma_xc(0, half=0)
        dma_w("wqh", wqh, 0)
        dma_w("wql", wql, 0)
        dma_xc(0, half=1)
        dma_w("wqh", wqh, 1)
        dma_w("wql", wql, 1)
        for h in range(2):
            dma_w("wkh", wkh, h)
            dma_w("wkl", wkl, h)
        for h in range(2):
            dma_w("wvh", wvh, h)
            dma_w("wvl", wvl, h)
        nc.gpsimd.dma_start(bq_t[:], bqt[:, :])
        nc.gpsimd.dma_start(mask_t[:], masks[:, :])
        nc.gpsimd.dma_start(ident_t[:], ident[:, :])
        nc.sync.dma_start(wo_t[:].rearrange("p (c j) -> p c j", c=NJ),
                          wo.rearrange("(c p) j -> p c j", p=128))

        # v pair block: [V_even(64) | ones(1) | V_odd(64) | pad(31)]
        vr = v_t[:].rearrange("p (s q w) -> p s q w", s=NST, q=4)
        nc.vector.memset(vr[:, :, :, DK:DK + 1], 1.0)

        # ---- phase B pieces: 12 matmul groups x 2 halves per s-chunk ----
        # fp8 DoubleRow with hi/lo error compensation: each dc-PAIR takes 3
        # DR matmuls — (w_hi,x_hi)+(w_hi,x_hi) slots over both dcs, then
        # (w_lo,x_hi) and (w_hi,x_lo); the dropped lo*lo term is ~1e-3.
        b_open = {}

        def b_dr(pb_slice, prods, jlo, jw, nlo, nw):
            """Accumulate a [jw(<=128) x nw(<=256)] psum slice over 4
            dc-pairs x 3 hi/lo products with DoubleRow matmuls."""
            n = 0
            for dcp in range(4):
                dc0 = 2 * dcp
                for lA, rA in prods:
                    n += 1
                    nc.tensor.matmul(
                        pb_slice,
                        lA[:, dc0:dc0 + 2, jlo:jlo + jw],
                        rA[:, dc0:dc0 + 2, nlo:nlo + nw],
                        start=(n == 1), stop=(n == 12),
                        perf_mode=DR)

        def b_group(sc, gi, half=None):
            """Group gi in 0..11: 0-3 qT j-tiles, 4-7 kT j-tiles, 8-11 V.
            Halves 0/1 each cover a 256-wide output strip (12 DR matmuls,
            ~640ns PE), sized to the per-exp-group ACT deficit."""
            if half is None:
                b_group(sc, gi, 0)
                b_group(sc, gi, 1)
                return
            xch, xcl = xc_tiles[sc]
            if half == 0:
                b_open[(sc, gi)] = pps.tile([128, 512], F32, tag="pp",
                                            name="pb")
            pb = b_open[(sc, gi)]
            if gi < 8:
                wn = "wq" if gi < 4 else "wk"
                wh = w_ts[wn + "h"][:].rearrange("p (c j) -> p c j", c=ND)
                wl = w_ts[wn + "l"][:].rearrange("p (c j) -> p c j", c=ND)
                dstT = qT_t if gi < 4 else kT_t
                jt = gi % 4
                prods = [(wh, xch), (wl, xch), (wh, xcl)]
                b_dr(pb[:, half * 256:(half + 1) * 256], prods,
                     jt * 128, 128, half * 256, 256)
                if half == 1:
                    dst = dstT[:, jt * S + sc * 512: jt * S + (sc + 1) * 512]
                    if gi < 4:
                        nc.vector.tensor_scalar(
                            dst, pb[:], bq_t[:, jt:jt + 1], None, op0=ADD)
                    else:
                        nc.vector.tensor_copy(dst, pb[:])
            else:
                st4 = gi - 8
                st = sc * 4 + st4
                wh = w_ts["wvh"][:].rearrange("p (c j) -> p c j", c=ND)
                wl = w_ts["wvl"][:].rearrange("p (c j) -> p c j", c=ND)
                prods = [(xch, wh), (xcl, wh), (xch, wl)]
                b_dr(pb[:, half * 256:(half + 1) * 256], prods,
                     st4 * 128, 128, half * 256, 256)
                if half == 1:
                    pv4 = pb[:].rearrange("p (q two w) -> p q two w",
                                          q=4, two=2)
                    nc.vector.tensor_copy(vr[:, st, :, 0:DK],
                                          pv4[:, :, 0, :])
                    nc.vector.tensor_copy(vr[:, st, :, VW:VW + DK],
                                          pv4[:, :, 1, :])
            if half == 1:
                del b_open[(sc, gi)]

        # Filler scheduling uses a coarse build-time clock model: est["pe"]
        # is cumulative emitted PE-busy ns, est["act"] the projected ACT
        # completion time.  Fillers are popped exactly when ACT runs ahead,
        # so foreign PE work lands in the exp-bound stretches.
        fillers = []
        est = {"credit": 0.0, "rate": 0.0}
        CYC = 0.4167

        done_keys = set()

        def pop_filler(force=False):
            if force:
                if fillers:
                    key, fn, pe_ns = fillers.pop(0)
                    fn()
                    done_keys.add(key)
                return
            est["credit"] += est["rate"]
            while fillers and est["credit"] >= 1.0:
                est["credit"] -= 1.0
                key, fn, pe_ns = fillers.pop(0)
                fn()
                done_keys.add(key)

        def require(*keys):
            """Force-emit queued fillers until all `keys` have run (data
            dependencies of the upcoming phase_c block)."""
            while fillers and not all(k in done_keys for k in keys):
                pop_filler(force=True)

        def phase_c(hp, qc):
            """Attention for head pair hp, q-chunk qc (256 wide).

            Scores land as e[k, q]; attnV is flipped: out po[q, V|1] with q
            on the full 128 partitions (lhsT = e 128-q slice, rhs =
            [V|ones] 65-wide moving).  The softmax denominator then lands
            on the same partition as its numerators, so normalization is a
            per-partition reciprocal+scale — no PE broadcast needed.

            The top diagonal k-tile (kt = n_k-1) only covers the second
            128-q half (its first half is fully causal-masked), so it gets
            a 128-wide slot; slots are packed tightly into 1024-wide psum
            groups with one exp per group.
            """
            n_k = 2 * qc + 2
            # Masked diagonal tiles FIRST so their DVE mask-multiply is off
            # the block's critical tail; psum accumulation is order-free.
            order = [n_k - 2, n_k - 1] + list(range(n_k - 2))
            # (kt, offset, width) slots packed greedily into 1024-wide
            # groups; within a group wide slots go first so no matmul
            # crosses a 512-f32 psum bank boundary.
            groups, cur, w_acc = [], [], 0
            for kt in order:
                w = 128 if kt == n_k - 1 else QW
                if w_acc + w > 1024:
                    groups.append(cur)
                    cur, w_acc = [], 0
                cur.append((kt, w))
                w_acc += w
            groups.append(cur)
            g2 = []
            for g in groups:
                g = sorted(g, key=lambda s: -s[1])
                off, withoff = 0, []
                for kt, w in g:
                    withoff.append((kt, off, w))
                    off += w
                g2.append(withoff)
            groups = g2
            # po accumulation flags follow EMISSION order, not kt order
            eseq = [kt for g in groups for (kt, _, _) in g]
            emit = {0: [kt for kt in eseq if kt != n_k - 1], 1: eseq}
            emit_first = {qs: emit[qs][0] for qs in range(2)}
            emit_last = {qs: emit[qs][-1] for qs in range(2)}
            for h2 in range(2):
                lo = h2 * 64
                po = [pat.tile([128, VW], F32, tag="po", name=f"po{qs}")
                      for qs in range(2)]

                def attn_v(g, e):
                    """Masks + attnV matmuls for a score group."""
                    for kt, off, w in g:
                        if kt == n_k - 2:  # diagonal tile, mask0
                            nc.vector.tensor_tensor(
                                e[:, off:off + QW], e[:, off:off + QW],
                                mask_t[:, 0:QW], op=MULT)
                        elif kt == n_k - 1:  # top tile: right half, mask128
                            nc.vector.tensor_tensor(
                                e[:, off:off + 128], e[:, off:off + 128],
                                mask_t[:, QW + 128:QW + 256], op=MULT)
                        base = kt * 4 * PW + hp * PW + h2 * DK
                        for qs in ((1,) if w == 128 else (0, 1)):
                            nc.tensor.matmul(
                                po[qs][:],
                                e[:, off + qs * 128 - (QW - w):
                                  off + qs * 128 - (QW - w) + 128],
                                v_t[:, base: base + VW],
                                start=(kt == emit_first[qs]),
                                stop=(kt == emit_last[qs]))

                for g in groups:
                    gw = g[-1][1] + g[-1][2]
                    ps = psc.tile([128, 1024], F32, tag="sc", name="ps")
                    for kt, off, w in g:
                        qoff = hp * S + qc * QW + (QW - w)
                        nc.tensor.matmul(
                            ps[:, off:off + w],
                            kT_t[lo:lo + 64,
                                 hp * S + kt * 128: hp * S + (kt + 1) * 128],
                            qT_t[lo:lo + 64, qoff: qoff + w],
                            start=True, stop=True)
                    e = ep.tile([128, 1024], BF16, tag="e", name="e")
                    nc.scalar.activation(e[:, :gw], ps[:, :gw],
                                         EXP, scale=0.125 / 1024)
                    pop_filler()  # PE fills while ACT runs the exp
                    attn_v(g, e)
                # normalize: denominator is col DK (h0) / col 0 (h1)
                dcol, ncol = (DK, 0) if h2 == 0 else (0, 1)
                for qs in range(2):
                    qt = qc * 2 + qs
                    rc = rp.tile([128, 1], F32, tag="rec", name="rc")
                    nc.vector.reciprocal(rc[:], po[qs][:, dcol:dcol + 1])
                    nc.vector.tensor_scalar(
                        o_sb[:, (qt * NJ + hp) * 128 + lo:
                             (qt * NJ + hp) * 128 + lo + DK],
                        po[qs][:, ncol:ncol + DK], rc[:], None, op0=MULT)
                pop_filler()

        d_tiles = {}

        def phase_d(st, piece=None):
            """Pieces 0-3: (od, half) quarters.  Piece 0 also transposes
            o[q, dv2] -> oT[dv2, q] per head pair (PE, bf16 identity)."""
            if piece is None:
                for p in range(4):
                    phase_d(st, p)
                return
            od, half = piece // 2, piece % 2
            if piece == 0:
                ot = ostp.tile([128, D], BF16, tag="ost", name="ot")
                oTst = otp.tile([128, NJ * 128], BF16, tag="oTst",
                                name="oTst")
                for hp in range(NJ):
                    tp = psc.tile([128, 128], BF16, tag="sc", name="tp")
                    nc.tensor.transpose(
                        tp[:],
                        o_sb[:, (st * NJ + hp) * 128:
                             (st * NJ + hp + 1) * 128],
                        ident_t[:])
                    nc.vector.tensor_copy(oTst[:, hp * 128:(hp + 1) * 128],
                                          tp[:])
                d_tiles[st] = (ot, oTst, {})
            ot, oTst, pds = d_tiles[st]
            if half == 0:
                pds[od] = pps.tile([128, 512], F32, tag="pp", name="pd")
            pd = pds[od]
            for vc in (range(2) if half == 0 else range(2, NJ)):
                nc.tensor.matmul(
                    pd[:],
                    oTst[:, vc * 128:(vc + 1) * 128],
                    wo_t[:, vc * D + od * 512: vc * D + (od + 1) * 512],
                    start=(vc == 0), stop=(vc == NJ - 1))
            if half == 1:
                nc.vector.tensor_copy(ot[:, od * 512:(od + 1) * 512], pd[:])
                del pds[od]
            if piece == 3:
                nc.sync.dma_start(out[st * 128:(st + 1) * 128, :], ot[:])
                del d_tiles[st]

        # ---- main loop: C(., sc) with B + D pieces as fillers ----
        # Each B(s) chunk (s>=1) splits: the groups C(., 2s) touches first
        # (qT0/kT0/V st0/st1) emit during sc=s-1; the rest defer into sc=s
        # behind require() guards, spreading PE work into the late,
        # exp-bound stretches.  D s-tiles are back-loaded similarly.
        d_sched = {0: [], 1: [], 2: [0, 1, 2, 3, 4, 5],
                   3: [6, 7, 8, 9, 10, 11]}
        b_defer = [1, 5, 10, 11, 2, 6, 3, 7]
        for gi in range(12):
            b_group(0, gi)
        # filler sites per sc: one per exp group + one per (h2, hp, qc)
        n_sites = {0: 32, 1: 48, 2: 64, 3: 80}
        for sc in range(NSC):
            if sc >= 1:
                for gi in b_defer:
                    for half in range(2):
                        fillers.append((
                            ("b", sc, gi, half),
                            lambda sc=sc, gi=gi, half=half:
                            b_group(sc, gi, half), 0))
            if sc < NSC - 1:
                dma_xc(sc + 1)
                for gi in [g for g in range(12) if g not in b_defer]:
                    for half in range(2):
                        fillers.append((
                            ("b", sc + 1, gi, half),
                            lambda sc=sc, gi=gi, half=half:
                            b_group(sc + 1, gi, half), 0))
            for st in d_sched[sc]:
                for piece in range(4):
                    fillers.append((
                        ("d", st, piece),
                        lambda st=st, piece=piece: phase_d(st, piece), 0))
            est["credit"] = 0.0
            est["rate"] = len(fillers) / n_sites[sc]
            for qci, qc in enumerate((2 * sc, 2 * sc + 1)):
                for hp in range(NJ):
                    if sc >= 1:
                        req = [("b", sc, g, q) for g in (hp, 4 + hp)
                               for q in range(2) if g in b_defer]
                        if qci == 1:
                            req += [("b", sc, g, q) for g in (10, 11)
                                    for q in range(2)]
                        require(*req)
                    phase_c(hp, qc)
                if sc == NSC - 1 and qci == 0:
                    # q-tiles 12/13 are final after the qc=6 pass: their
                    # phase D becomes filler for the last C stretch
                    for st in (12, 13):
                        for piece in range(4):
                            fillers.append((
                                ("d", st, piece),
                                lambda st=st, piece=piece:
                                phase_d(st, piece), 0))
                    est["rate"] = len(fillers) / 40
            while fillers:
                pop_filler(force=True)
        for st4 in range(2):
            phase_d(14 + st4)

    nc.compile()
    return nc


# ----------------------------------------------------------------- host side

_NC_CACHE = None


def _get_nc():
    global _NC_CACHE
    if _NC_CACHE is None:
        _NC_CACHE = build_mha()
    return _NC_CACHE


def make_masks():
    """[128, 512]: two diagonal masks for [128k x 256q] tiles.
    mask0: k-tile aligned with q-chunk start (keep k<=q: i<=j).
    mask128: k-tile offset +128 (keep i+128<=j)."""
    i = np.arange(128)[:, None]
    j = np.arange(QW)[None, :]
    m0 = (i <= j).astype(np.float32)
    m128 = (i + 128 <= j).astype(np.float32)
    return np.concatenate([m0, m128], axis=1)


def split_fp8(a):
    """hi/lo decomposition: a ~= hi + lo with both in fp8e4m3."""
    import ml_dtypes
    f8 = ml_dtypes.float8_e4m3fn
    hi = a.astype(f8)
    lo = (a - hi.astype(np.float32)).astype(f8)
    return np.ascontiguousarray(hi), np.ascontiguousarray(lo)


def shard_inputs(x, Wq, bq, Wk, bk, Wv, bv, Wo, bo):
    import ml_dtypes
    masks = make_masks().astype(ml_dtypes.bfloat16)
    ident = np.eye(128, dtype=np.float32).astype(ml_dtypes.bfloat16)
    x = np.asarray(x, dtype=np.float32)
    Wq, Wk, Wv, Wo = (np.asarray(a, dtype=np.float32)
                      for a in (Wq, Wk, Wv, Wo))
    bq = np.asarray(bq, dtype=np.float32)
    maps = []
    for c in range(8):
        b, g = c // 2, c % 2
        sl = slice(g * JC, (g + 1) * JC)
        # bq per-core slice laid out [128 part, jt]: j = jt*128 + p
        # weights are pre-scaled x32 so their fp8 hi/lo split avoids the
        # e4m3 subnormal floor; 1/1024 folds into the exp scale and 1/32
        # into wo (the ones-column denominator is unscaled, so o_sb is x32).
        bqt = np.ascontiguousarray(
            bq[sl].reshape(NJ, 128).T).astype(np.float32) * 32.0
        xh, xl = split_fp8(np.ascontiguousarray(x[b].T))
        wqh, wql = split_fp8(Wq[:, sl] * 32.0)
        wkh, wkl = split_fp8(Wk[:, sl] * 32.0)
        wvh, wvl = split_fp8(Wv[:, sl] * 32.0)
        maps.append({
            "xh": xh, "xl": xl,
            "wqh": wqh, "wql": wql,
            "wkh": wkh, "wkl": wkl,
            "wvh": wvh, "wvl": wvl,
            "wo": np.ascontiguousarray(Wo[sl, :] / 32.0).astype(
                ml_dtypes.bfloat16),
            "bqt": bqt,
            "masks": masks,
            "ident": ident,
        })
    return maps


def kernel(x, Wq, bq, Wk, bk, Wv, bv, Wo, bo):
    """Full-input entry point: shard across 8 NeuronCores, run, gather."""
    from concourse.bass_utils import run_bass_kernel_spmd

    nc = _get_nc()
    in_maps = shard_inputs(x, Wq, bq, Wk, bk, Wv, bv, Wo, bo)
    res = run_bass_kernel_spmd(nc, in_maps, list(range(8)))
    # bv contributes bv @ Wo to every output row (softmax weights sum to 1)
    bo_eff = (np.asarray(bo, dtype=np.float32)
              + np.asarray(bv, dtype=np.float32)
              @ np.asarray(Wo, dtype=np.float32))
    out = np.empty((4, S, D), dtype=np.float32)
    for b in range(4):
        out[b] = (res.results[2 * b]["out"].astype(np.float32)
                  + res.results[2 * b + 1]["out"].astype(np.float32)
                  + bo_eff)
    return out
